# revision 1
# baseline (speedup 1.0000x reference)
"""Bass/Trainium2 kernel for nn_EnhancedBianGuaAttention_76055280878201.

Contract: kernel(**inputs) takes the FULL unsharded inputs (as produced by
reference.setup_inputs()) and returns the FULL (B, T, D) output.

Sharding: 8 cores = 2 batches x 4 head-groups (4 heads each).  Each core:
  - computes q/k/v projections (transposed layout) for its 4 heads from x[b]^T
  - computes u^T = tanh([hex_w; q6_w] @ x^T)  (12, T) and per-head
    A_h^T = B_h^T u^T where B_h = diag(lam/2 * I6, sig(scale)*2/6 * Mh)
    so that the full score bias is bias[i,j] = sum_c u[j,c] A_h[i,c]
  - flash-style causal attention, scores computed transposed (keys on the
    partition axis):  E^T[j,i] = exp(alpha*tanh(qk/beta) + bias^T),
    masked with affine_select; denominators via a ones-column appended to V
  - its 4 heads' slice of the output projection, written transposed
Host side: sums the 4 partial outputs per batch and transposes.

Precision: the projection/score/AV chains run in SC_DT (bf16 by default; PE
full rate, cheap ACT/DVE, half DMA).  The output projection and the softmax
normalization stay in fp32r/fp32 (PSUM accumulation is always fp32).
"""

import os
import sys

import numpy as np

for _p in ("/opt/trn_rl_repo", "/root/.axon_site/_ro/trn_rl_repo"):
    if os.path.isdir(_p) and _p not in sys.path:
        sys.path.append(_p)

import ml_dtypes
import concourse.bacc as bacc
import concourse.mybir as mybir
import concourse.tile as tile
from concourse.bass_utils import run_bass_kernel_spmd

B, T, D, H, NT = 2, 2048, 1024, 16, 7
HD = D // H          # 64
TEMP = 0.5
NCORES = 8
HPC = 4              # heads per core
CPB = NCORES // B    # cores per batch (4)
TC = 512             # query-chunk size
NTC = T // TC        # 4
JB = 128             # key-block size
NJB = T // JB        # 16
KC = D // 128        # contraction chunks for the projections (8)

F32 = mybir.dt.float32
F32R = mybir.dt.float32r
BF16 = mybir.dt.bfloat16
Act = mybir.ActivationFunctionType
Alu = mybir.AluOpType

# score-chain dtype: BF16 (fast) or F32R (precise)
USE_BF16 = os.environ.get("KERNEL_F32R") != "1"


def _emit(nc, tc_, dr, alpha, inv_beta):
    """Emit the per-core program. dr: dict of DRAM APs."""
    SC = BF16 if USE_BF16 else F32R
    xT_r = dr["xT"].rearrange("(c p) t -> c p t", p=128)       # (8,128,T)
    wqkv_r = dr["wqkvT"].rearrange("(c p) m -> c p m", p=128)  # (8,128,768)
    whq_r = dr["whqT"].rearrange("(c p) w -> p c w", p=128)    # (128,8,12)

    with (
        tc_.tile_pool(name="persist", bufs=1) as pp,
        tc_.tile_pool(name="work", bufs=1) as wp,
        tc_.tile_pool(name="psum", bufs=1, space="PSUM") as sp,
    ):
        # ---- constants / weights (pre-rounded on host) --------------
        ident = pp.tile([128, 128], SC)
        nc.sync.dma_start(out=ident[:], in_=dr["ident"])
        ones64 = pp.tile([1, 64], F32R)
        nc.sync.dma_start(out=ones64[:], in_=dr["ones64"])
        bTz = pp.tile([128, 12 * HPC], SC)
        nc.gpsimd.memset(bTz[:], 0.0) if USE_BF16 else nc.vector.memset(bTz[:], 0.0)
        nc.sync.dma_start(out=bTz[0:12, :], in_=dr["bT"])
        owt = [pp.tile([128, D], F32R, name=f"owt{i}") for i in range(2)]
        for i in range(2):
            nc.sync.dma_start(out=owt[i][:], in_=dr["owT"][i * 128:(i + 1) * 128, :])
        whq = wp.tile([128, KC, 12], SC)
        nc.sync.dma_start(out=whq[:], in_=whq_r)
        wq = [wp.tile([128, 3 * HPC * HD], SC, name=f"wq{c}") for c in range(KC)]
        for c in range(KC):
            nc.sync.dma_start(out=wq[c][:], in_=wqkv_r[c])
        ones_col = pp.tile([128, 1], F32)
        nc.gpsimd.memset(ones_col[:], 1.0)
        tri = pp.tile([128, 128], SC)
        nc.sync.dma_start(out=tri[:], in_=dr["tri"])

        # ---- persistent activations ---------------------------------
        # All score-chain matmuls are zero-padded to a full K=128
        # contraction: the PE clock-gate (HAM) watches array activity, and
        # low-K matmuls read as idle and get the clock halved.
        _ms = nc.gpsimd if USE_BF16 else nc.vector
        uz = pp.tile([128, T], SC)
        _ms.memset(uz[:], 0.0)
        aT = [pp.tile([128, T], SC, name=f"aT{h}") for h in range(HPC)]
        for h in range(HPC):
            _ms.memset(aT[h][:], 0.0)
        # q zero-padded per head (other head's 64 rows are 0);
        # k keeps 2 heads per tile: rows h%2*64 .. +64
        qz = [pp.tile([128, T], SC, name=f"qz{h}") for h in range(HPC)]
        for h in range(HPC):
            _ms.memset(qz[h][:], 0.0)
        kt = [pp.tile([128, T], SC, name=f"kt{i}") for i in range(2)]
        vt = [pp.tile([128, T], SC, name=f"vt{i}") for i in range(2)]
        qkv_tiles = [None, None] + kt + vt  # oc: q01,q23,k01,k23,v01,v23
        # V' per head: natural layout + ones column, 16 blocks of (128, 65)
        vp = [pp.tile([128, NJB * (HD + 1)], SC, name=f"vp{h}")
              for h in range(HPC)]
        # normalized attention out (transposed, f32r), 2 heads per tile
        ao = [pp.tile([128, T], F32R, name=f"ao{i}") for i in range(2)]
        # K=128-padded denominator-broadcast operands (kills the K=1
        # matmuls that re-throttle the PE clock gate around each norm)
        e0z = pp.tile([128, HD], F32R)
        nc.sync.dma_start(out=e0z[:], in_=dr["e0z"])
        dnz = [pp.tile([128, TC], F32R, name=f"dnz{i}") for i in range(2)]
        for i in range(2):
            nc.sync.dma_start(out=dnz[i][:], in_=dr["zz"])

        # ---- per-t-chunk projection + V'-build emitters -------------
        def proj_chunk(t4):
            sl = slice(t4 * TC, (t4 + 1) * TC)
            xt = [
                wp.tile([128, TC], SC, name=f"xt{c}", tag=f"xt{c}", bufs=2)
                for c in range(KC)
            ]
            for c in range(KC):
                nc.sync.dma_start(out=xt[c][:], in_=xT_r[c, :, sl])

            # u^T chunk
            pu = sp.tile([12, TC], F32, name="pu", tag="mm", bufs=5)
            for c in range(KC):
                nc.tensor.matmul(pu[:], whq[:, c, :], xt[c][:],
                                 start=(c == 0), stop=(c == KC - 1))
            nc.scalar.activation(uz[0:12, sl], pu[:], Act.Tanh)

            # A_h^T chunks (off the attention hot path)
            for h in range(HPC):
                pa = sp.tile([12, TC], F32, name="pa", tag="mm", bufs=5)
                nc.tensor.matmul(pa[:], bTz[:, 12 * h:12 * h + 12],
                                 uz[:, sl], start=True, stop=True)
                nc.vector.tensor_copy(aT[h][0:12, sl], pa[:])

            # qkv projections; q chunks are split into per-head
            # zero-padded tiles (see K=128 note above)
            for oc in range(6):
                pq = sp.tile([128, TC], F32, name="pq", tag="mm", bufs=5)
                for c in range(KC):
                    nc.tensor.matmul(pq[:], wq[c][:, oc * 128:(oc + 1) * 128],
                                     xt[c][:], start=(c == 0), stop=(c == KC - 1))
                if oc < 2:
                    nc.vector.tensor_copy(qz[2 * oc][0:HD, sl], pq[0:HD, :])
                    nc.vector.tensor_copy(qz[2 * oc + 1][HD:128, sl],
                                          pq[HD:128, :])
                else:
                    nc.vector.tensor_copy(qkv_tiles[oc][:, sl], pq[:])

        def vp_build(t4):
            for h in range(HPC):
                ro = (h % 2) * HD
                for tb in range(4 * t4, 4 * t4 + 4):
                    pv = sp.tile([128, HD], SC, name="pv", tag="mm", bufs=5)
                    nc.tensor.transpose(
                        pv[:], vt[h // 2][ro:ro + HD, tb * JB:(tb + 1) * JB],
                        ident[ro:ro + HD, ro:ro + HD])
                    nc.vector.tensor_copy(
                        vp[h][:, tb * (HD + 1):tb * (HD + 1) + HD], pv[:])
                    nc.vector.tensor_copy(
                        vp[h][:, tb * (HD + 1) + HD:(tb + 1) * (HD + 1)],
                        ones_col[:])

        # ---- attention (ic outer, head-pairs interleaved) -----------
        # Two independent (h, ic) dependency chains in flight keep every
        # engine fed; the output projection for t-chunk ic is emitted as
        # soon as all 4 heads finish that ic, overlapping phase D with C.
        def attn_tile(h, ic, jb, po, pair):
            # Full (non-diagonal) tiles arrive in pairs sharing a (128, 2*TC)
            # t1/ee tile so the exp runs once per pair (halves ACT op
            # overhead).  pair = (t1p, eep, member) or None for diagonal
            # tiles, which process only their live columns [off, TC).
            njb = 4 * (ic + 1)
            jsl = slice(jb * JB, (jb + 1) * JB)
            off = max(0, jb * JB - ic * TC)
            w = TC - off
            csl = slice(ic * TC + off, (ic + 1) * TC)
            pr = sp.tile([128, TC], F32, name="pr", tag="mm", bufs=5)
            nc.tensor.matmul(pr[:, :w], kt[h // 2][:, jsl], qz[h][:, csl],
                             start=True, stop=True)
            if pair is None:
                t1 = wp.tile([128, TC], SC, name="t1", tag="t1", bufs=8)
                ee = wp.tile([128, TC], SC, name="ee", tag="ee", bufs=8)
                t1v, eev = t1[:, :w], ee[:, :w]
            else:
                t1p, eep, q = pair
                qsl = slice(q * TC, (q + 1) * TC)
                t1v, eev = t1p[:, qsl], eep[:, qsl]
            nc.scalar.activation(t1v, pr[:, :w], Act.Tanh, scale=inv_beta)
            # bias matmul reuses the SAME psum slot (write-after-read on the
            # tanh): halves PSUM pressure per tile -> twice the tiles in
            # flight
            nc.tensor.matmul(pr[:, :w], uz[:, jsl], aT[h][:, csl],
                             start=True, stop=True)
            nc.vector.scalar_tensor_tensor(
                t1v, t1v, alpha, pr[:, :w], op0=Alu.mult, op1=Alu.add)
            if pair is None:
                nc.scalar.activation(eev, t1v, Act.Exp)
                if jb >= 4 * ic:
                    nc.vector.tensor_mul(ee[:, 0:JB], ee[:, 0:JB], tri[:])
                attn_av(h, ic, jb, po, eev)
            elif pair[2] == 1:
                nc.scalar.activation(eep[:], t1p[:], Act.Exp)

        def attn_av(h, ic, jb, po, eev):
            njb = 4 * (ic + 1)
            off = max(0, jb * JB - ic * TC)
            nc.tensor.matmul(
                po[:, off:], vp[h][:, jb * (HD + 1):(jb + 1) * (HD + 1)],
                eev, start=(jb == 0), stop=(jb == njb - 1))

        def attn_norm(h, ic, po):
            # normalize rows 0..63 by 1/row64: broadcast the denominator
            # down 64 partitions via a rank-1 matmul, then a fast
            # reciprocal on all 64 lanes, then one multiply.
            ro = (h % 2) * HD
            isl = slice(ic * TC, (ic + 1) * TC)
            dz = dnz[(h * NTC + ic) % 2]
            nc.scalar.copy(dz[0:1, :], po[HD:HD + 1, :])
            prb = sp.tile([HD, TC], F32, name="prb", tag="pf", bufs=1)
            nc.tensor.matmul(prb[:], e0z[:], dz[:], start=True, stop=True)
            rb = wp.tile([HD, TC], F32, name="rb", tag="rb", bufs=2)
            nc.vector.reciprocal_approx_fast(rb[:], prb[:])
            nc.vector.tensor_mul(ao[h // 2][ro:ro + HD, isl],
                                 po[0:HD, :], rb[:])

        # The t4=ic+1 projection chunk is emitted right after attention(ic)
        # so the scheduler can pull its matmuls forward into attention's PE
        # stall gaps (keeps the PE dense and the HAM clock-gate open).
        proj_chunk(0)
        vp_build(0)
        for ic in range(NTC):
            njb = 4 * (ic + 1)
            sl = slice(ic * TC, (ic + 1) * TC)
            if ic + 1 < NTC:
                proj_chunk(ic + 1)
                vp_build(ic + 1)
            for hp in (0, 2):
                po_a = sp.tile([HD + 1, TC], F32, name="po_a", tag="po", bufs=2)
                po_b = sp.tile([HD + 1, TC], F32, name="po_b", tag="po", bufs=2)
                for g in range(2 * ic):
                    t1p_a = wp.tile([128, 2 * TC], SC, name="t1p_a",
                                    tag="t1p", bufs=4)
                    eep_a = wp.tile([128, 2 * TC], SC, name="eep_a",
                                    tag="eep", bufs=4)
                    t1p_b = wp.tile([128, 2 * TC], SC, name="t1p_b",
                                    tag="t1p", bufs=4)
                    eep_b = wp.tile([128, 2 * TC], SC, name="eep_b",
                                    tag="eep", bufs=4)
                    attn_tile(hp, ic, 2 * g, po_a, (t1p_a, eep_a, 0))
                    attn_tile(hp + 1, ic, 2 * g, po_b, (t1p_b, eep_b, 0))
                    attn_tile(hp, ic, 2 * g + 1, po_a, (t1p_a, eep_a, 1))
                    attn_tile(hp + 1, ic, 2 * g + 1, po_b, (t1p_b, eep_b, 1))
                    attn_av(hp, ic, 2 * g, po_a, eep_a[:, 0:TC])
                    attn_av(hp + 1, ic, 2 * g, po_b, eep_b[:, 0:TC])
                    attn_av(hp, ic, 2 * g + 1, po_a, eep_a[:, TC:2 * TC])
                    attn_av(hp + 1, ic, 2 * g + 1, po_b, eep_b[:, TC:2 * TC])
                for q in range(4):
                    attn_tile(hp, ic, 4 * ic + q, po_a, None)
                    attn_tile(hp + 1, ic, 4 * ic + q, po_b, None)
                attn_norm(hp, ic, po_a)
                attn_norm(hp + 1, ic, po_b)
            # output projection for this t-chunk (all heads now done)
            for ec in range(D // 128):
                esl = slice(ec * 128, (ec + 1) * 128)
                pf = sp.tile([128, TC], F32, name="pf", tag="pf", bufs=1)
                nc.tensor.matmul(pf[:], owt[0][:, esl], ao[0][:, sl],
                                 start=True, stop=False)
                nc.tensor.matmul(pf[:], owt[1][:, esl], ao[1][:, sl],
                                 start=False, stop=True)
                fo = wp.tile([128, TC], F32, name="fo", tag="fo", bufs=3)
                nc.scalar.copy(fo[:], pf[:])
                nc.sync.dma_start(out=dr["poutT"][esl, sl], in_=fo[:])


def _build(alpha, inv_beta):
    SC = BF16 if USE_BF16 else F32R
    nc = bacc.Bacc("TRN2", debug=False)
    dr = {}
    dr["xT"] = nc.dram_tensor("xT", [D, T], SC, kind="ExternalInput").ap()
    dr["wqkvT"] = nc.dram_tensor(
        "wqkvT", [D, 3 * HPC * HD], SC, kind="ExternalInput").ap()
    dr["whqT"] = nc.dram_tensor("whqT", [D, 12], SC, kind="ExternalInput").ap()
    dr["bT"] = nc.dram_tensor("bT", [12, 12 * HPC], SC, kind="ExternalInput").ap()
    dr["owT"] = nc.dram_tensor(
        "owT", [HPC * HD, D], F32R, kind="ExternalInput").ap()
    dr["ident"] = nc.dram_tensor("ident", [128, 128], SC, kind="ExternalInput").ap()
    dr["tri"] = nc.dram_tensor("tri", [128, 128], SC, kind="ExternalInput").ap()
    dr["ones64"] = nc.dram_tensor("ones64", [1, 64], F32R, kind="ExternalInput").ap()
    dr["e0z"] = nc.dram_tensor("e0z", [128, HD], F32R, kind="ExternalInput").ap()
    dr["zz"] = nc.dram_tensor("zz", [128, TC], F32R, kind="ExternalInput").ap()
    dr["poutT"] = nc.dram_tensor("poutT", [D, T], F32, kind="ExternalOutput").ap()
    with tile.TileContext(nc) as tc_:
        _emit(nc, tc_, dr, alpha, inv_beta)
    nc.compile()
    return nc


def _sigmoid(v):
    return 1.0 / (1.0 + np.exp(-v))


def _round_f32r(a):
    """Round fp32 -> fp32r bit pattern (11-bit mantissa, rte)."""
    u = np.ascontiguousarray(a, np.float32).view(np.uint32)
    r = (u + 0x7FF + ((u >> 12) & 1)) & np.uint32(0xFFFFF000)
    return r.view(np.float32)


def _sc_cast(a):
    """Cast an fp32 array to the score-chain wire dtype."""
    a = np.ascontiguousarray(a, np.float32)
    if USE_BF16:
        return a.astype(ml_dtypes.bfloat16)
    return _round_f32r(a)


def _host_prep(x, qkv_w, out_w, hex_w, hamming_lambda_logit, q6_w,
               transforms, transform_weights, scale_logit, sips_alpha,
               sips_beta):
    """Build the per-core input maps (all host work is slicing/transposes)."""
    x = np.asarray(x, np.float32)
    qkv_w = np.asarray(qkv_w, np.float32)
    out_w = np.asarray(out_w, np.float32)
    hex_w = np.asarray(hex_w, np.float32)
    q6_w = np.asarray(q6_w, np.float32)
    transforms = np.asarray(transforms, np.float32)
    transform_weights = np.asarray(transform_weights, np.float32)

    lam = float(_sigmoid(np.float32(hamming_lambda_logit)))
    scale2 = float(_sigmoid(np.float32(scale_logit))) * 2.0
    alpha = float(np.asarray(sips_alpha).reshape(-1)[0])
    inv_beta = 1.0 / float(np.asarray(sips_beta).reshape(-1)[0])

    tw = np.asarray(transform_weights, np.float64) / TEMP
    w = np.exp(tw - tw.max(-1, keepdims=True))
    w = (w / w.sum(-1, keepdims=True)).astype(np.float32)      # (H, NT)
    Mh = np.einsum("ht,tde->hde", w, transforms)               # (H, 6, 6)

    whqT = _sc_cast(np.vstack([hex_w, q6_w]).T)                # (D, 12)
    ident = _sc_cast(np.eye(128, dtype=np.float32))
    # tri[p, f] = 1 if f >= p (keep) else 0 -- diagonal-block causal mask
    tri = _sc_cast((np.arange(128)[None, :] >= np.arange(128)[:, None])
                   .astype(np.float32))
    ones64 = np.ones((1, HD), np.float32)
    e0z_h = np.zeros((128, HD), np.float32); e0z_h[0, :] = 1.0
    zz_h = np.zeros((128, TC), np.float32)
    bigB = np.zeros((H, 12, 12), np.float32)
    for h in range(H):
        bigB[h, :6, :6] = (lam / 2.0) * np.eye(6, dtype=np.float32)
        bigB[h, 6:, 6:] = (scale2 / 6.0) * Mh[h]

    in_maps = []
    for core in range(NCORES):
        b = core // CPB
        heads = [(core % CPB) * HPC + k for k in range(HPC)]
        rows = []
        for part in range(3):
            for h in heads:
                rows.extend(range(part * D + h * HD, part * D + (h + 1) * HD))
        wqkvT = _sc_cast(qkv_w[rows, :].T)                      # (D, 768)
        cols = []
        for h in heads:
            cols.extend(range(h * HD, (h + 1) * HD))
        owT = _round_f32r(out_w[:, cols].T)                     # (256, D)
        bT = np.concatenate([bigB[h].T for h in heads], axis=1)  # (12, 48)
        in_maps.append({
            "xT": _sc_cast(x[b].T),
            "wqkvT": wqkvT,
            "whqT": whqT,
            "bT": _sc_cast(bT),
            "owT": owT,
            "ident": ident,
            "ones64": ones64,
            "e0z": e0z_h,
            "zz": zz_h,
            "tri": tri,
        })
    return in_maps, alpha, inv_beta


_CACHE = {}
LAST_RESULT = None


def kernel(**inputs):
    global LAST_RESULT
    in_maps, alpha, inv_beta = _host_prep(**inputs)
    key = (round(alpha, 9), round(inv_beta, 9), USE_BF16)
    if key not in _CACHE:
        _CACHE[key] = _build(alpha, inv_beta)
    nc = _CACHE[key]
    res = run_bass_kernel_spmd(nc, in_maps, list(range(NCORES)))
    LAST_RESULT = res
    out = np.zeros((B, T, D), np.float32)
    for b in range(B):
        acc = np.zeros((D, T), np.float32)
        for core in range(b * CPB, (b + 1) * CPB):
            acc += res.results[core]["poutT"]
        out[b] = acc.T
    return out



# revision 2
# speedup vs baseline: 1.0229x; 1.0229x over previous
"""Bass/Trainium2 kernel for nn_EnhancedBianGuaAttention_76055280878201, v3.

Contract: kernel(**inputs) takes the FULL unsharded inputs (as produced by
reference.setup_inputs()) and returns the FULL (B, T, D) output.

Sharding: 8 cores = 2 batches x 4 head-groups (4 heads each).

v3 notes: all score-chain matmuls are K=128 zero-padded (HW tracing of a
K=64 row-packed variant showed the PE HAM clock gate oscillating to
K=4/8 for ~40% of the kernel; padded-K keeps it at 8/8).  A head-pair's
two score tiles share one [128, 1024] PSUM tile so tanh/add/exp run as
single FD~1024 ops.  V' tiles are prefilled with 1.0 so the denominator
ones-column needs no writes.  Dummy matmuls + a dummy activation at the
start warm the PE clock gate and pull the ACT table load into the
initial DMA wait.  Output is written bf16 (summed in fp32 on host).
"""

import os
import sys

import numpy as np

for _p in ("/opt/trn_rl_repo", "/root/.axon_site/_ro/trn_rl_repo"):
    if os.path.isdir(_p) and _p not in sys.path:
        sys.path.append(_p)

import ml_dtypes
import concourse.bacc as bacc
import concourse.mybir as mybir
import concourse.tile as tile
from concourse.bass_utils import run_bass_kernel_spmd

B, T, D, H, NT = 2, 2048, 1024, 16, 7
HD = D // H          # 64
TEMP = 0.5
NCORES = 8
HPC = 4              # heads per core
CPB = NCORES // B    # cores per batch (4)
TC = 512             # query-chunk size
NTC = T // TC        # 4
JB = 128             # key-block size
NJB = T // JB        # 16
KC = D // 128        # contraction chunks for the projections (8)

F32 = mybir.dt.float32
F32R = mybir.dt.float32r
BF16 = mybir.dt.bfloat16
Act = mybir.ActivationFunctionType
Alu = mybir.AluOpType


def _emit(nc, tc_, dr, alpha, inv_beta):
    SC = BF16
    xT_r = dr["xT"].rearrange("(c p) (f t) -> c p f t", p=128, f=NTC)  # (8,128,4,512)
    wqkv_r = dr["wqkvT"].rearrange("(c p) m -> c p m", p=128)  # (8,128,768)
    whq_r = dr["whqT"].rearrange("(c p) w -> p c w", p=128)    # (128,8,12)

    with (
        tc_.tile_pool(name="persist", bufs=1) as pp,
        tc_.tile_pool(name="work", bufs=1) as wp,
        tc_.tile_pool(name="psum", bufs=1, space="PSUM") as sp,
    ):
        # ---- PE/ACT warmup (no DMA dependency): keeps the HAM clock
        # gate warm and pulls the ACT table load into the DMA wait.
        dumb = pp.tile([128, TC], SC)
        nc.gpsimd.memset(dumb[:], 0.0)
        nc.scalar.activation(dumb[:, 0:128], dumb[:, 0:128], Act.Exp)
        for i in range(20):
            pw = sp.tile([128, TC], F32, name="pw", tag="pr", bufs=2)
            nc.tensor.matmul(pw[:], dumb[:, 0:128], dumb[:],
                             start=True, stop=True)

        # ---- weights / constants; DMA priority: x+whq+wq first, owt last
        xt = [pp.tile([128, T], SC, name=f"xt{c}") for c in range(KC)]
        for c in range(KC):
            nc.sync.dma_start(out=xt[c][:, 0:TC], in_=xT_r[c, :, 0])
        whq = pp.tile([128, KC, 12], SC)
        nc.sync.dma_start(out=whq[:], in_=whq_r)
        wq = [pp.tile([128, 3 * HPC * HD], SC, name=f"wq{c}") for c in range(KC)]
        for c in range(KC):
            nc.sync.dma_start(out=wq[c][:], in_=wqkv_r[c])
        bTz = pp.tile([128, 12 * HPC], SC)
        nc.gpsimd.memset(bTz[:], 0.0)
        nc.sync.dma_start(out=bTz[0:12, :], in_=dr["bT"])
        ident = pp.tile([128, 128], SC)
        nc.sync.dma_start(out=ident[:], in_=dr["ident"])
        tri = pp.tile([128, 128], SC)
        nc.sync.dma_start(out=tri[:], in_=dr["tri"])
        e0z = pp.tile([128, HD], F32R)
        nc.sync.dma_start(out=e0z[:], in_=dr["e0z"])
        dnz = [pp.tile([128, TC], F32R, name=f"dnz{i}") for i in range(2)]
        for i in range(2):
            nc.sync.dma_start(out=dnz[i][:], in_=dr["zz"])
        for c in range(KC):
            for t4 in range(1, NTC):
                nc.sync.dma_start(out=xt[c][:, t4 * TC:(t4 + 1) * TC],
                                  in_=xT_r[c, :, t4])
        owt = [pp.tile([128, D], F32R, name=f"owt{i}") for i in range(2)]
        for i in range(2):
            nc.sync.dma_start(out=owt[i][:], in_=dr["owT"][i * 128:(i + 1) * 128, :])

        # ---- persistent activations ---------------------------------
        # k/v: pair i holds head 2i at partitions 0:64, head 2i+1 at 64:128
        # q: per-head zero-padded (the other 64 partitions are 0) so the
        # QK matmul runs full K=128 against the pair's k tile
        kt = [pp.tile([128, T], SC, name=f"kt{i}") for i in range(2)]
        qz = [pp.tile([128, T], SC, name=f"qz{h}") for h in range(HPC)]
        for h in range(HPC):
            nc.gpsimd.memset(qz[h][:], 0.0)
        # u^T at partitions 0:12, zero elsewhere (K=128-padded bias lhsT)
        uz = pp.tile([128, T], SC)
        nc.gpsimd.memset(uz[:], 0.0)
        # A_h^T per head at partitions 0:12 (K=128-padded bias rhs)
        aT = [pp.tile([128, T], SC, name=f"aT{h}") for h in range(HPC)]
        for h in range(HPC):
            nc.gpsimd.memset(aT[h][:], 0.0)
        # V': (128 keys, head, block, 64 d + 1 ones); prefill 1.0 so the
        # denominator ones-column never needs writing
        vp_all = pp.tile([128, HPC, NJB, HD + 1], SC)
        nc.gpsimd.memset(vp_all[:], 1.0)
        # normalized attention out (transposed, f32r), pair-shared
        ao = [pp.tile([128, T], F32R, name=f"ao{i}") for i in range(2)]

        # ---- projection packets for one t-chunk ---------------------
        def proj_packets(t4):
            sl = slice(t4 * TC, (t4 + 1) * TC)

            def p_u():
                pu = sp.tile([12, TC], F32, name="pu", tag="mm", bufs=2)
                for c in range(KC):
                    nc.tensor.matmul(pu[:], whq[:, c, :], xt[c][:, sl],
                                     start=(c == 0), stop=(c == KC - 1))
                nc.scalar.activation(uz[0:12, sl], pu[:], Act.Tanh)

            def p_a():
                for h in range(HPC):
                    pa = sp.tile([12, TC], F32, name="pa", tag="mm", bufs=2)
                    nc.tensor.matmul(pa[:], bTz[:, 12 * h:12 * h + 12],
                                     uz[:, sl], start=True, stop=True)
                    nc.vector.tensor_copy(aT[h][0:12, sl], pa[:])

            def mk_qkv(oc):
                def p_qkv():
                    pq = sp.tile([128, TC], F32, name="pq", tag="mm", bufs=2)
                    for c in range(KC):
                        nc.tensor.matmul(pq[:], wq[c][:, oc * 128:(oc + 1) * 128],
                                         xt[c][:, sl], start=(c == 0),
                                         stop=(c == KC - 1))
                    if oc < 2:
                        nc.vector.tensor_copy(qz[2 * oc][0:HD, sl], pq[0:HD, :])
                        nc.scalar.copy(qz[2 * oc + 1][HD:128, sl], pq[HD:128, :])
                    else:
                        nc.vector.tensor_copy(kt[oc - 2][:, sl], pq[:])
                return p_qkv

            def mk_vt(k):
                # V^T computed directly: out[t, hd] = sum_d x[t, d] wv[d, hd]
                # (lhsT = x^T block, moving = the v columns of wqkv)
                def p_vt():
                    tb = 4 * t4 + k
                    tsl = slice(tb * JB, (tb + 1) * JB)
                    pv = sp.tile([128, HPC * HD], F32, name="pv", tag="mm",
                                 bufs=2)
                    for c in range(KC):
                        nc.tensor.matmul(pv[:], xt[c][:, tsl],
                                         wq[c][:, 512:768],
                                         start=(c == 0), stop=(c == KC - 1))
                    nc.vector.tensor_copy(
                        vp_all[:, :, tb, 0:HD],
                        pv[:].rearrange("p (h c) -> p h c", c=HD))
                return p_vt

            pk = [p_u, mk_qkv(0), mk_qkv(1), mk_qkv(2), mk_qkv(3), p_a]
            pk += [mk_vt(k) for k in range(4)]
            return pk

        # ---- attention row for one (chunk, head-pair) ---------------
        def attn_row(ic, i, feed):
            he, ho = 2 * i, 2 * i + 1
            njb = 4 * (ic + 1)
            isl = slice(ic * TC, (ic + 1) * TC)
            po_e = sp.tile([HD + 1, TC], F32, name="po_e", tag="po", bufs=2)
            po_o = sp.tile([HD + 1, TC], F32, name="po_o", tag="po", bufs=2)
            for jb in range(njb):
                if feed:
                    feed()
                jsl = slice(jb * JB, (jb + 1) * JB)
                off = max(0, jb * JB - ic * TC)
                csl = slice(ic * TC + off, (ic + 1) * TC)
                pr = sp.tile([128, 2 * TC], F32, name="pr", tag="pr", bufs=2)
                nc.tensor.matmul(pr[:, off:TC], kt[i][:, jsl],
                                 qz[he][:, csl], start=True, stop=True)
                nc.tensor.matmul(pr[:, TC + off:2 * TC], kt[i][:, jsl],
                                 qz[ho][:, csl], start=True, stop=True)
                t2 = wp.tile([128, 2 * TC], SC, name="t2", tag="t2", bufs=6)
                nc.scalar.activation(t2[:, off:2 * TC], pr[:, off:2 * TC],
                                     Act.Tanh, scale=inv_beta)
                # bias matmuls reuse the PSUM slots (WAR on the tanh)
                nc.tensor.matmul(pr[:, off:TC], uz[:, jsl],
                                 aT[he][:, csl], start=True, stop=True)
                nc.tensor.matmul(pr[:, TC + off:2 * TC], uz[:, jsl],
                                 aT[ho][:, csl], start=True, stop=True)
                nc.vector.scalar_tensor_tensor(
                    t2[:, off:2 * TC], t2[:, off:2 * TC], alpha,
                    pr[:, off:2 * TC], op0=Alu.mult, op1=Alu.add)
                ee = wp.tile([128, 2 * TC], SC, name="ee", tag="ee", bufs=6)
                nc.scalar.activation(ee[:, off:2 * TC], t2[:, off:2 * TC],
                                     Act.Exp)
                if jb >= 4 * ic:  # diagonal block: causal mask (gpsimd --
                    # the only engine with cycles to spare here)
                    nc.gpsimd.tensor_mul(ee[:, off:off + JB],
                                         ee[:, off:off + JB], tri[:])
                    nc.gpsimd.tensor_mul(ee[:, TC + off:TC + off + JB],
                                         ee[:, TC + off:TC + off + JB],
                                         tri[:])
                nc.tensor.matmul(po_e[:, off:], vp_all[:, he, jb, :],
                                 ee[:, off:TC],
                                 start=(jb == 0), stop=(jb == njb - 1))
                nc.tensor.matmul(po_o[:, off:], vp_all[:, ho, jb, :],
                                 ee[:, TC + off:2 * TC],
                                 start=(jb == 0), stop=(jb == njb - 1))
            # normalize: broadcast denominator row down 64 partitions via
            # rank-1 matmul, reciprocal, multiply
            for k, po in ((0, po_e), (1, po_o)):
                dz = dnz[k]
                nc.vector.tensor_copy(dz[0:1, :], po[HD:HD + 1, :])
                prb = sp.tile([HD, TC], F32, name="prb", tag="mm", bufs=2)
                nc.tensor.matmul(prb[:], e0z[:], dz[:], start=True, stop=True)
                rb = wp.tile([HD, TC], F32, name="rb", tag="rb", bufs=2)
                nc.vector.reciprocal_approx_fast(rb[:], prb[:])
                nc.vector.tensor_mul(ao[i][k * HD:(k + 1) * HD, isl],
                                     po[0:HD, :], rb[:])

        def outproj(ic):
            sl = slice(ic * TC, (ic + 1) * TC)
            for ec in range(D // 128):
                esl = slice(ec * 128, (ec + 1) * 128)
                pf = sp.tile([128, TC], F32, name="pf", tag="mm", bufs=2)
                nc.tensor.matmul(pf[:], owt[0][:, esl], ao[0][:, sl],
                                 start=True, stop=False)
                nc.tensor.matmul(pf[:], owt[1][:, esl], ao[1][:, sl],
                                 start=False, stop=True)
                fo = wp.tile([128, TC], SC, name="fo", tag="fo", bufs=4)
                if ec % 2 == 0:
                    nc.scalar.copy(fo[:], pf[:])
                else:
                    nc.vector.tensor_copy(fo[:], pf[:])
                nc.sync.dma_start(out=dr["poutT"][esl, sl], in_=fo[:])

        # ---- main schedule ------------------------------------------
        # proj packets are emitted AFTER the attention rows they overlap
        # with: lower scheduler priority, so the critical attention chain
        # owns the engines and proj fills the gaps.
        for f in proj_packets(0):
            f()
        for ic in range(NTC):
            pending = list(proj_packets(ic + 1)) if ic + 1 < NTC else []
            if ic > 0:
                pending.append(lambda ic=ic: outproj(ic - 1))
            it = iter(pending)

            def feed():
                f = next(it, None)
                if f:
                    f()
            attn_row(ic, 0, feed)
            attn_row(ic, 1, feed)
            for f in it:
                f()
        outproj(NTC - 1)


def _build(alpha, inv_beta):
    nc = bacc.Bacc("TRN2", debug=False)
    dr = {}
    dr["xT"] = nc.dram_tensor("xT", [D, T], BF16, kind="ExternalInput").ap()
    dr["wqkvT"] = nc.dram_tensor(
        "wqkvT", [D, 3 * HPC * HD], BF16, kind="ExternalInput").ap()
    dr["whqT"] = nc.dram_tensor("whqT", [D, 12], BF16, kind="ExternalInput").ap()
    dr["bT"] = nc.dram_tensor("bT", [12, 48], BF16, kind="ExternalInput").ap()
    dr["owT"] = nc.dram_tensor(
        "owT", [HPC * HD, D], F32R, kind="ExternalInput").ap()
    dr["ident"] = nc.dram_tensor("ident", [128, 128], BF16, kind="ExternalInput").ap()
    dr["tri"] = nc.dram_tensor("tri", [128, 128], BF16, kind="ExternalInput").ap()
    dr["e0z"] = nc.dram_tensor("e0z", [128, HD], F32R, kind="ExternalInput").ap()
    dr["zz"] = nc.dram_tensor("zz", [128, TC], F32R, kind="ExternalInput").ap()
    dr["poutT"] = nc.dram_tensor("poutT", [D, T], BF16, kind="ExternalOutput").ap()
    with tile.TileContext(nc) as tc_:
        _emit(nc, tc_, dr, alpha, inv_beta)
    nc.compile()
    return nc


def _sigmoid(v):
    return 1.0 / (1.0 + np.exp(-v))


def _round_f32r(a):
    """Round fp32 -> fp32r bit pattern (11-bit mantissa, rte)."""
    u = np.ascontiguousarray(a, np.float32).view(np.uint32)
    r = (u + 0x7FF + ((u >> 12) & 1)) & np.uint32(0xFFFFF000)
    return r.view(np.float32)


def _bf(a):
    return np.ascontiguousarray(a, np.float32).astype(ml_dtypes.bfloat16)


def _host_prep(x, qkv_w, out_w, hex_w, hamming_lambda_logit, q6_w,
               transforms, transform_weights, scale_logit, sips_alpha,
               sips_beta):
    x = np.asarray(x, np.float32)
    qkv_w = np.asarray(qkv_w, np.float32)
    out_w = np.asarray(out_w, np.float32)
    hex_w = np.asarray(hex_w, np.float32)
    q6_w = np.asarray(q6_w, np.float32)
    transforms = np.asarray(transforms, np.float32)
    transform_weights = np.asarray(transform_weights, np.float32)

    lam = float(_sigmoid(np.float32(hamming_lambda_logit)))
    scale2 = float(_sigmoid(np.float32(scale_logit))) * 2.0
    alpha = float(np.asarray(sips_alpha).reshape(-1)[0])
    inv_beta = 1.0 / float(np.asarray(sips_beta).reshape(-1)[0])

    tw = np.asarray(transform_weights, np.float64) / TEMP
    w = np.exp(tw - tw.max(-1, keepdims=True))
    w = (w / w.sum(-1, keepdims=True)).astype(np.float32)      # (H, NT)
    Mh = np.einsum("ht,tde->hde", w, transforms)               # (H, 6, 6)

    whqT = _bf(np.vstack([hex_w, q6_w]).T)                     # (D, 12)
    ident = _bf(np.eye(128, dtype=np.float32))
    tri = _bf((np.arange(128)[None, :] >= np.arange(128)[:, None])
              .astype(np.float32))
    e0z_h = np.zeros((128, HD), np.float32); e0z_h[0, :] = 1.0
    bigB = np.zeros((H, 12, 12), np.float32)
    for h in range(H):
        bigB[h, :6, :6] = (lam / 2.0) * np.eye(6, dtype=np.float32)
        bigB[h, 6:, 6:] = (scale2 / 6.0) * Mh[h]

    in_maps = []
    for core in range(NCORES):
        b = core // CPB
        heads = [(core % CPB) * HPC + k for k in range(HPC)]
        rows = []
        for part in range(3):
            for h in heads:
                rows.extend(range(part * D + h * HD, part * D + (h + 1) * HD))
        wqkvT = _bf(qkv_w[rows, :].T)                          # (D, 768)
        cols = []
        for h in heads:
            cols.extend(range(h * HD, (h + 1) * HD))
        owT = _round_f32r(out_w[:, cols].T)                    # (256, D)
        bT = np.concatenate([bigB[h].T for h in heads], axis=1)  # (12, 48)
        in_maps.append({
            "xT": _bf(x[b].T),
            "wqkvT": wqkvT,
            "whqT": whqT,
            "bT": _bf(bT),
            "owT": owT,
            "ident": ident,
            "e0z": e0z_h,
            "zz": np.zeros((128, TC), np.float32),
            "tri": tri,
        })
    return in_maps, alpha, inv_beta


_CACHE = {}
LAST_RESULT = None


def kernel(**inputs):
    global LAST_RESULT
    in_maps, alpha, inv_beta = _host_prep(**inputs)
    key = (round(alpha, 9), round(inv_beta, 9))
    if key not in _CACHE:
        _CACHE[key] = _build(alpha, inv_beta)
    nc = _CACHE[key]
    res = run_bass_kernel_spmd(nc, in_maps, list(range(NCORES)))
    LAST_RESULT = res
    out = np.zeros((B, T, D), np.float32)
    for b in range(B):
        acc = np.zeros((D, T), np.float32)
        for core in range(b * CPB, (b + 1) * CPB):
            acc += np.asarray(res.results[core]["poutT"], np.float32)
        out[b] = acc.T
    return out


# revision 3
# speedup vs baseline: 1.0273x; 1.0043x over previous
"""Bass/Trainium2 kernel for nn_EnhancedBianGuaAttention_76055280878201, v3.

Contract: kernel(**inputs) takes the FULL unsharded inputs (as produced by
reference.setup_inputs()) and returns the FULL (B, T, D) output.

Sharding: 8 cores = 2 batches x 4 head-groups (4 heads each).

v3 notes: all score-chain matmuls are K=128 zero-padded (HW tracing of a
K=64 row-packed variant showed the PE HAM clock gate oscillating to
K=4/8 for ~40% of the kernel; padded-K keeps it at 8/8).  A head-pair's
two score tiles share one [128, 1024] PSUM tile so tanh/add/exp run as
single FD~1024 ops.  V' tiles are prefilled with 1.0 so the denominator
ones-column needs no writes.  Dummy matmuls + a dummy activation at the
start warm the PE clock gate and pull the ACT table load into the
initial DMA wait.  Output is written bf16 (summed in fp32 on host).
"""

import os
import sys

import numpy as np

for _p in ("/opt/trn_rl_repo", "/root/.axon_site/_ro/trn_rl_repo"):
    if os.path.isdir(_p) and _p not in sys.path:
        sys.path.append(_p)

import ml_dtypes
import concourse.bacc as bacc
import concourse.mybir as mybir
import concourse.tile as tile
from concourse.bass_utils import run_bass_kernel_spmd

B, T, D, H, NT = 2, 2048, 1024, 16, 7
HD = D // H          # 64
TEMP = 0.5
NCORES = 8
HPC = 4              # heads per core
CPB = NCORES // B    # cores per batch (4)
TC = 512             # query-chunk size
NTC = T // TC        # 4
JB = 128             # key-block size
NJB = T // JB        # 16
KC = D // 128        # contraction chunks for the projections (8)

F32 = mybir.dt.float32
F32R = mybir.dt.float32r
BF16 = mybir.dt.bfloat16
Act = mybir.ActivationFunctionType
Alu = mybir.AluOpType


def _emit(nc, tc_, dr, alpha, inv_beta):
    SC = BF16
    xT_r = dr["xT"].rearrange("(c p) (f t) -> c p f t", p=128, f=NTC)  # (8,128,4,512)
    wqkv_r = dr["wqkvT"].rearrange("(c p) m -> c p m", p=128)  # (8,128,768)
    whq_r = dr["whqT"].rearrange("(c p) w -> p c w", p=128)    # (128,8,12)

    with (
        tc_.tile_pool(name="persist", bufs=1) as pp,
        tc_.tile_pool(name="work", bufs=1) as wp,
        tc_.tile_pool(name="psum", bufs=1, space="PSUM") as sp,
    ):
        # ---- PE/ACT warmup (no DMA dependency): keeps the HAM clock
        # gate warm and pulls the ACT table load into the DMA wait.
        dumb = pp.tile([128, TC], SC)
        nc.gpsimd.memset(dumb[:], 0.0)
        nc.scalar.activation(dumb[:, 0:128], dumb[:, 0:128], Act.Exp)
        for i in range(20):
            pw = sp.tile([128, TC], F32, name="pw", tag="pr", bufs=2)
            nc.tensor.matmul(pw[:], dumb[:, 0:128], dumb[:],
                             start=True, stop=True)

        # ---- weights / constants; DMA priority: x+whq+wq first, owt last
        xt = [pp.tile([128, T], SC, name=f"xt{c}") for c in range(KC)]
        for c in range(KC):
            nc.sync.dma_start(out=xt[c][:, 0:TC], in_=xT_r[c, :, 0])
        whq = pp.tile([128, KC, 12], SC)
        nc.sync.dma_start(out=whq[:], in_=whq_r)
        wq = [pp.tile([128, 3 * HPC * HD], SC, name=f"wq{c}") for c in range(KC)]
        for c in range(KC):
            nc.sync.dma_start(out=wq[c][:], in_=wqkv_r[c])
        bTz = pp.tile([128, 12 * HPC], SC)
        nc.gpsimd.memset(bTz[:], 0.0)
        nc.sync.dma_start(out=bTz[0:12, :], in_=dr["bT"])
        ident = pp.tile([128, 128], SC)
        nc.sync.dma_start(out=ident[:], in_=dr["ident"])
        tri = pp.tile([128, 128], SC)
        nc.sync.dma_start(out=tri[:], in_=dr["tri"])
        e0z = pp.tile([128, HD], F32R)
        nc.sync.dma_start(out=e0z[:], in_=dr["e0z"])
        dnz = [pp.tile([128, TC], F32R, name=f"dnz{i}") for i in range(2)]
        for i in range(2):
            nc.sync.dma_start(out=dnz[i][:], in_=dr["zz"])
        for c in range(KC):
            for t4 in range(1, NTC):
                nc.sync.dma_start(out=xt[c][:, t4 * TC:(t4 + 1) * TC],
                                  in_=xT_r[c, :, t4])
        owt = [pp.tile([128, D], F32R, name=f"owt{i}") for i in range(2)]
        for i in range(2):
            nc.sync.dma_start(out=owt[i][:], in_=dr["owT"][i * 128:(i + 1) * 128, :])

        # ---- persistent activations ---------------------------------
        # k/v: pair i holds head 2i at partitions 0:64, head 2i+1 at 64:128
        # q: per-head zero-padded (the other 64 partitions are 0) so the
        # QK matmul runs full K=128 against the pair's k tile
        kt = [pp.tile([128, T], SC, name=f"kt{i}") for i in range(2)]
        qz = [pp.tile([128, T], SC, name=f"qz{h}") for h in range(HPC)]
        for h in range(HPC):
            nc.gpsimd.memset(qz[h][:], 0.0)
        # u^T at partitions 0:12, zero elsewhere (K=128-padded bias lhsT)
        uz = pp.tile([128, T], SC)
        nc.gpsimd.memset(uz[:], 0.0)
        # A_h^T per head at partitions 0:12 (K=128-padded bias rhs)
        aT = [pp.tile([128, T], SC, name=f"aT{h}") for h in range(HPC)]
        for h in range(HPC):
            nc.gpsimd.memset(aT[h][:], 0.0)
        # V': (128 keys, head, block, 64 d + 1 ones); prefill 1.0 so the
        # denominator ones-column never needs writing
        vp_all = pp.tile([128, HPC, NJB, HD + 1], SC)
        nc.gpsimd.memset(vp_all[:], 1.0)
        # normalized attention out (transposed, f32r), pair-shared
        ao = [pp.tile([128, T], F32R, name=f"ao{i}") for i in range(2)]

        # ---- projection packets for one t-chunk ---------------------
        def proj_packets(t4):
            sl = slice(t4 * TC, (t4 + 1) * TC)

            def p_u():
                pu = sp.tile([12, TC], F32, name="pu", tag="mm", bufs=2)
                for c in range(KC):
                    nc.tensor.matmul(pu[:], whq[:, c, :], xt[c][:, sl],
                                     start=(c == 0), stop=(c == KC - 1))
                def fin():
                    nc.scalar.activation(uz[0:12, sl], pu[:], Act.Tanh)
                return fin

            def p_a():
                ps, fins = [], []
                for h in range(HPC):
                    pa = sp.tile([12, TC], F32, name=f"pa{h % 2}", tag="mm",
                                 bufs=2)
                    nc.tensor.matmul(pa[:], bTz[:, 12 * h:12 * h + 12],
                                     uz[:, sl], start=True, stop=True)
                    def fin(h=h, pa=pa):
                        nc.vector.tensor_copy(aT[h][0:12, sl], pa[:])
                    if h % 2 == 1:
                        fins.append(fin)
                    else:
                        fin()
                def fin_all():
                    for f in fins:
                        f()
                return fin_all

            def mk_qkv(oc):
                def p_qkv():
                    pq = sp.tile([128, TC], F32, name="pq", tag="mm", bufs=2)
                    for c in range(KC):
                        nc.tensor.matmul(pq[:], wq[c][:, oc * 128:(oc + 1) * 128],
                                         xt[c][:, sl], start=(c == 0),
                                         stop=(c == KC - 1))
                    def fin():
                        if oc < 2:
                            nc.vector.tensor_copy(qz[2 * oc][0:HD, sl],
                                                  pq[0:HD, :])
                            nc.scalar.copy(qz[2 * oc + 1][HD:128, sl],
                                           pq[HD:128, :])
                        else:
                            nc.vector.tensor_copy(kt[oc - 2][:, sl], pq[:])
                    return fin
                return p_qkv

            def mk_vt(k):
                # V^T computed directly: out[t, hd] = sum_d x[t, d] wv[d, hd]
                # (lhsT = x^T block, moving = the v columns of wqkv)
                def p_vt():
                    tb = 4 * t4 + k
                    tsl = slice(tb * JB, (tb + 1) * JB)
                    pv = sp.tile([128, HPC * HD], F32, name="pv", tag="mm",
                                 bufs=2)
                    for c in range(KC):
                        nc.tensor.matmul(pv[:], xt[c][:, tsl],
                                         wq[c][:, 512:768],
                                         start=(c == 0), stop=(c == KC - 1))
                    def fin():
                        nc.vector.tensor_copy(
                            vp_all[:, :, tb, 0:HD],
                            pv[:].rearrange("p (h c) -> p h c", c=HD))
                    return fin
                return p_vt

            pk = [p_u, mk_qkv(0), mk_qkv(1), mk_qkv(2), mk_qkv(3), p_a]
            pk += [mk_vt(k) for k in range(4)]
            return pk

        # ---- attention row for one (chunk, head-pair) ---------------
        def attn_row(ic, i, feed):
            he, ho = 2 * i, 2 * i + 1
            njb = 4 * (ic + 1)
            isl = slice(ic * TC, (ic + 1) * TC)
            po_e = sp.tile([HD + 1, TC], F32, name="po_e", tag="po", bufs=2)
            po_o = sp.tile([HD + 1, TC], F32, name="po_o", tag="po", bufs=2)
            for jb in range(njb):
                if feed:
                    feed()
                jsl = slice(jb * JB, (jb + 1) * JB)
                off = max(0, jb * JB - ic * TC)
                csl = slice(ic * TC + off, (ic + 1) * TC)
                pr = sp.tile([128, 2 * TC], F32, name="pr", tag="pr", bufs=2)
                nc.tensor.matmul(pr[:, off:TC], kt[i][:, jsl],
                                 qz[he][:, csl], start=True, stop=True)
                nc.tensor.matmul(pr[:, TC + off:2 * TC], kt[i][:, jsl],
                                 qz[ho][:, csl], start=True, stop=True)
                t2 = wp.tile([128, 2 * TC], SC, name="t2", tag="t2", bufs=6)
                nc.scalar.activation(t2[:, off:2 * TC], pr[:, off:2 * TC],
                                     Act.Tanh, scale=inv_beta)
                # bias matmuls reuse the PSUM slots (WAR on the tanh)
                nc.tensor.matmul(pr[:, off:TC], uz[:, jsl],
                                 aT[he][:, csl], start=True, stop=True)
                nc.tensor.matmul(pr[:, TC + off:2 * TC], uz[:, jsl],
                                 aT[ho][:, csl], start=True, stop=True)
                nc.vector.scalar_tensor_tensor(
                    t2[:, off:2 * TC], t2[:, off:2 * TC], alpha,
                    pr[:, off:2 * TC], op0=Alu.mult, op1=Alu.add)
                ee = wp.tile([128, 2 * TC], SC, name="ee", tag="ee", bufs=6)
                nc.scalar.activation(ee[:, off:2 * TC], t2[:, off:2 * TC],
                                     Act.Exp)
                if jb >= 4 * ic:  # diagonal block: causal mask (gpsimd --
                    # the only engine with cycles to spare here)
                    nc.gpsimd.tensor_mul(ee[:, off:off + JB],
                                         ee[:, off:off + JB], tri[:])
                    nc.gpsimd.tensor_mul(ee[:, TC + off:TC + off + JB],
                                         ee[:, TC + off:TC + off + JB],
                                         tri[:])
                nc.tensor.matmul(po_e[:, off:], vp_all[:, he, jb, :],
                                 ee[:, off:TC],
                                 start=(jb == 0), stop=(jb == njb - 1))
                nc.tensor.matmul(po_o[:, off:], vp_all[:, ho, jb, :],
                                 ee[:, TC + off:2 * TC],
                                 start=(jb == 0), stop=(jb == njb - 1))
            # normalize: broadcast denominator row down 64 partitions via
            # rank-1 matmul, reciprocal, multiply
            for k, po in ((0, po_e), (1, po_o)):
                dz = dnz[k]
                nc.vector.tensor_copy(dz[0:1, :], po[HD:HD + 1, :])
                prb = sp.tile([HD, TC], F32, name="prb", tag="mm", bufs=2)
                nc.tensor.matmul(prb[:], e0z[:], dz[:], start=True, stop=True)
                rb = wp.tile([HD, TC], F32, name="rb", tag="rb", bufs=2)
                nc.vector.reciprocal_approx_fast(rb[:], prb[:])
                nc.vector.tensor_mul(ao[i][k * HD:(k + 1) * HD, isl],
                                     po[0:HD, :], rb[:])

        def outproj(ic):
            sl = slice(ic * TC, (ic + 1) * TC)
            for ec in range(D // 128):
                esl = slice(ec * 128, (ec + 1) * 128)
                pf = sp.tile([128, TC], F32, name="pf", tag="mm", bufs=2)
                nc.tensor.matmul(pf[:], owt[0][:, esl], ao[0][:, sl],
                                 start=True, stop=False)
                nc.tensor.matmul(pf[:], owt[1][:, esl], ao[1][:, sl],
                                 start=False, stop=True)
                fo = wp.tile([128, TC], SC, name="fo", tag="fo", bufs=4)
                if ec % 2 == 0:
                    nc.scalar.copy(fo[:], pf[:])
                else:
                    nc.vector.tensor_copy(fo[:], pf[:])
                nc.sync.dma_start(out=dr["poutT"][esl, sl], in_=fo[:])

        # ---- main schedule ------------------------------------------
        # proj packets are emitted AFTER the attention rows they overlap
        # with: lower scheduler priority, so the critical attention chain
        # owns the engines and proj fills the gaps.
        for f in proj_packets(0):
            fin = f()
            if fin:
                fin()
        for ic in range(NTC):
            pending = list(proj_packets(ic + 1)) if ic + 1 < NTC else []
            if ic > 0:
                pending.append(lambda ic=ic: outproj(ic - 1))
            it = iter(pending)
            held = [None]

            def feed():
                if held[0]:
                    held[0]()
                    held[0] = None
                f = next(it, None)
                if f:
                    held[0] = f()
            attn_row(ic, 0, feed)
            attn_row(ic, 1, feed)
            if held[0]:
                held[0]()
            for f in it:
                fin = f()
                if fin:
                    fin()
        outproj(NTC - 1)


def _build(alpha, inv_beta):
    nc = bacc.Bacc("TRN2", debug=False)
    dr = {}
    dr["xT"] = nc.dram_tensor("xT", [D, T], BF16, kind="ExternalInput").ap()
    dr["wqkvT"] = nc.dram_tensor(
        "wqkvT", [D, 3 * HPC * HD], BF16, kind="ExternalInput").ap()
    dr["whqT"] = nc.dram_tensor("whqT", [D, 12], BF16, kind="ExternalInput").ap()
    dr["bT"] = nc.dram_tensor("bT", [12, 48], BF16, kind="ExternalInput").ap()
    dr["owT"] = nc.dram_tensor(
        "owT", [HPC * HD, D], F32R, kind="ExternalInput").ap()
    dr["ident"] = nc.dram_tensor("ident", [128, 128], BF16, kind="ExternalInput").ap()
    dr["tri"] = nc.dram_tensor("tri", [128, 128], BF16, kind="ExternalInput").ap()
    dr["e0z"] = nc.dram_tensor("e0z", [128, HD], F32R, kind="ExternalInput").ap()
    dr["zz"] = nc.dram_tensor("zz", [128, TC], F32R, kind="ExternalInput").ap()
    dr["poutT"] = nc.dram_tensor("poutT", [D, T], BF16, kind="ExternalOutput").ap()
    with tile.TileContext(nc) as tc_:
        _emit(nc, tc_, dr, alpha, inv_beta)
    nc.compile()
    return nc


def _sigmoid(v):
    return 1.0 / (1.0 + np.exp(-v))


def _round_f32r(a):
    """Round fp32 -> fp32r bit pattern (11-bit mantissa, rte)."""
    u = np.ascontiguousarray(a, np.float32).view(np.uint32)
    r = (u + 0x7FF + ((u >> 12) & 1)) & np.uint32(0xFFFFF000)
    return r.view(np.float32)


def _bf(a):
    return np.ascontiguousarray(a, np.float32).astype(ml_dtypes.bfloat16)


def _host_prep(x, qkv_w, out_w, hex_w, hamming_lambda_logit, q6_w,
               transforms, transform_weights, scale_logit, sips_alpha,
               sips_beta):
    x = np.asarray(x, np.float32)
    qkv_w = np.asarray(qkv_w, np.float32)
    out_w = np.asarray(out_w, np.float32)
    hex_w = np.asarray(hex_w, np.float32)
    q6_w = np.asarray(q6_w, np.float32)
    transforms = np.asarray(transforms, np.float32)
    transform_weights = np.asarray(transform_weights, np.float32)

    lam = float(_sigmoid(np.float32(hamming_lambda_logit)))
    scale2 = float(_sigmoid(np.float32(scale_logit))) * 2.0
    alpha = float(np.asarray(sips_alpha).reshape(-1)[0])
    inv_beta = 1.0 / float(np.asarray(sips_beta).reshape(-1)[0])

    tw = np.asarray(transform_weights, np.float64) / TEMP
    w = np.exp(tw - tw.max(-1, keepdims=True))
    w = (w / w.sum(-1, keepdims=True)).astype(np.float32)      # (H, NT)
    Mh = np.einsum("ht,tde->hde", w, transforms)               # (H, 6, 6)

    whqT = _bf(np.vstack([hex_w, q6_w]).T)                     # (D, 12)
    ident = _bf(np.eye(128, dtype=np.float32))
    tri = _bf((np.arange(128)[None, :] >= np.arange(128)[:, None])
              .astype(np.float32))
    e0z_h = np.zeros((128, HD), np.float32); e0z_h[0, :] = 1.0
    bigB = np.zeros((H, 12, 12), np.float32)
    for h in range(H):
        bigB[h, :6, :6] = (lam / 2.0) * np.eye(6, dtype=np.float32)
        bigB[h, 6:, 6:] = (scale2 / 6.0) * Mh[h]

    in_maps = []
    for core in range(NCORES):
        b = core // CPB
        heads = [(core % CPB) * HPC + k for k in range(HPC)]
        rows = []
        for part in range(3):
            for h in heads:
                rows.extend(range(part * D + h * HD, part * D + (h + 1) * HD))
        wqkvT = _bf(qkv_w[rows, :].T)                          # (D, 768)
        cols = []
        for h in heads:
            cols.extend(range(h * HD, (h + 1) * HD))
        owT = _round_f32r(out_w[:, cols].T)                    # (256, D)
        bT = np.concatenate([bigB[h].T for h in heads], axis=1)  # (12, 48)
        in_maps.append({
            "xT": _bf(x[b].T),
            "wqkvT": wqkvT,
            "whqT": whqT,
            "bT": _bf(bT),
            "owT": owT,
            "ident": ident,
            "e0z": e0z_h,
            "zz": np.zeros((128, TC), np.float32),
            "tri": tri,
        })
    return in_maps, alpha, inv_beta


_CACHE = {}
LAST_RESULT = None


def kernel(**inputs):
    global LAST_RESULT
    in_maps, alpha, inv_beta = _host_prep(**inputs)
    key = (round(alpha, 9), round(inv_beta, 9))
    if key not in _CACHE:
        _CACHE[key] = _build(alpha, inv_beta)
    nc = _CACHE[key]
    res = run_bass_kernel_spmd(nc, in_maps, list(range(NCORES)))
    LAST_RESULT = res
    out = np.zeros((B, T, D), np.float32)
    for b in range(B):
        acc = np.zeros((D, T), np.float32)
        for core in range(b * CPB, (b + 1) * CPB):
            acc += np.asarray(res.results[core]["poutT"], np.float32)
        out[b] = acc.T
    return out


# revision 4
# speedup vs baseline: 1.0283x; 1.0009x over previous
"""Bass/Trainium2 kernel for nn_EnhancedBianGuaAttention_76055280878201, v3.

Contract: kernel(**inputs) takes the FULL unsharded inputs (as produced by
reference.setup_inputs()) and returns the FULL (B, T, D) output.

Sharding: 8 cores = 2 batches x 4 head-groups (4 heads each).

v3 notes: all score-chain matmuls are K=128 zero-padded (HW tracing of a
K=64 row-packed variant showed the PE HAM clock gate oscillating to
K=4/8 for ~40% of the kernel; padded-K keeps it at 8/8).  A head-pair's
two score tiles share one [128, 1024] PSUM tile so tanh/add/exp run as
single FD~1024 ops.  V' tiles are prefilled with 1.0 so the denominator
ones-column needs no writes.  Dummy matmuls + a dummy activation at the
start warm the PE clock gate and pull the ACT table load into the
initial DMA wait.  Output is written bf16 (summed in fp32 on host).
"""

import os
import sys

import numpy as np

for _p in ("/opt/trn_rl_repo", "/root/.axon_site/_ro/trn_rl_repo"):
    if os.path.isdir(_p) and _p not in sys.path:
        sys.path.append(_p)

import ml_dtypes
import concourse.bacc as bacc
import concourse.mybir as mybir
import concourse.tile as tile
from concourse.bass_utils import run_bass_kernel_spmd

B, T, D, H, NT = 2, 2048, 1024, 16, 7
HD = D // H          # 64
TEMP = 0.5
NCORES = 8
HPC = 4              # heads per core
CPB = NCORES // B    # cores per batch (4)
TC = 512             # query-chunk size
NTC = T // TC        # 4
JB = 128             # key-block size
NJB = T // JB        # 16
KC = D // 128        # contraction chunks for the projections (8)

F32 = mybir.dt.float32
F32R = mybir.dt.float32r
BF16 = mybir.dt.bfloat16
Act = mybir.ActivationFunctionType
Alu = mybir.AluOpType


def _emit(nc, tc_, dr, alpha, inv_beta):
    SC = BF16
    xT_r = dr["xT"].rearrange("(c p) (f t) -> c p f t", p=128, f=NTC)  # (8,128,4,512)
    wqkv_r = dr["wqkvT"].rearrange("(c p) m -> c p m", p=128)  # (8,128,768)
    whq_r = dr["whqT"].rearrange("(c p) w -> p c w", p=128)    # (128,8,12)

    with (
        tc_.tile_pool(name="persist", bufs=1) as pp,
        tc_.tile_pool(name="work", bufs=1) as wp,
        tc_.tile_pool(name="psum", bufs=1, space="PSUM") as sp,
    ):
        # ---- PE/ACT warmup (no DMA dependency): keeps the HAM clock
        # gate warm and pulls the ACT table load into the DMA wait.
        dumb = pp.tile([128, TC], SC)
        nc.gpsimd.memset(dumb[:], 0.0)
        nc.scalar.activation(dumb[:, 0:128], dumb[:, 0:128], Act.Exp)
        for i in range(20):
            pw = sp.tile([128, TC], F32, name="pw", tag="pr", bufs=2)
            nc.tensor.matmul(pw[:], dumb[:, 0:128], dumb[:],
                             start=True, stop=True)

        # ---- weights / constants; DMA priority: x+whq+wq first, owt last
        xt = [pp.tile([128, T], SC, name=f"xt{c}") for c in range(KC)]
        for c in range(KC):
            nc.sync.dma_start(out=xt[c][:, 0:TC], in_=xT_r[c, :, 0])
        whq = pp.tile([128, KC, 12], SC)
        nc.sync.dma_start(out=whq[:], in_=whq_r)
        wq = [pp.tile([128, 3 * HPC * HD], SC, name=f"wq{c}") for c in range(KC)]
        for c in range(KC):
            nc.sync.dma_start(out=wq[c][:], in_=wqkv_r[c])
        bTz = pp.tile([128, 12 * HPC], SC)
        nc.gpsimd.memset(bTz[:], 0.0)
        nc.sync.dma_start(out=bTz[0:12, :], in_=dr["bT"])
        ident = pp.tile([128, 128], SC)
        nc.sync.dma_start(out=ident[:], in_=dr["ident"])
        tri = pp.tile([128, 128], SC)
        nc.sync.dma_start(out=tri[:], in_=dr["tri"])
        e0z = pp.tile([128, HD], F32R)
        nc.sync.dma_start(out=e0z[:], in_=dr["e0z"])
        dnz = [pp.tile([128, TC], F32R, name=f"dnz{i}") for i in range(2)]
        for i in range(2):
            nc.sync.dma_start(out=dnz[i][:], in_=dr["zz"])
        for c in range(KC):
            for t4 in range(1, NTC):
                nc.sync.dma_start(out=xt[c][:, t4 * TC:(t4 + 1) * TC],
                                  in_=xT_r[c, :, t4])
        owt = [pp.tile([128, D], F32R, name=f"owt{i}") for i in range(2)]
        for i in range(2):
            nc.sync.dma_start(out=owt[i][:], in_=dr["owT"][i * 128:(i + 1) * 128, :])

        # ---- persistent activations ---------------------------------
        # k/v: pair i holds head 2i at partitions 0:64, head 2i+1 at 64:128
        # q: per-head zero-padded (the other 64 partitions are 0) so the
        # QK matmul runs full K=128 against the pair's k tile
        kt = [pp.tile([128, T], SC, name=f"kt{i}") for i in range(2)]
        qz = [pp.tile([128, T], SC, name=f"qz{h}") for h in range(HPC)]
        for h in range(HPC):
            nc.gpsimd.memset(qz[h][:], 0.0)
        # u^T at partitions 0:12, zero elsewhere (K=128-padded bias lhsT)
        uz = pp.tile([128, T], SC)
        nc.gpsimd.memset(uz[:], 0.0)
        # A_h^T per head at partitions 0:12 (K=128-padded bias rhs)
        aT = [pp.tile([128, T], SC, name=f"aT{h}") for h in range(HPC)]
        for h in range(HPC):
            nc.gpsimd.memset(aT[h][:], 0.0)
        # V': (128 keys, head, block, 64 d + 1 ones); prefill 1.0 so the
        # denominator ones-column never needs writing
        vp_all = pp.tile([128, HPC, NJB, HD + 1], SC)
        nc.gpsimd.memset(vp_all[:], 1.0)
        # normalized attention out (transposed, f32r), pair-shared
        ao = [pp.tile([128, T], F32R, name=f"ao{i}") for i in range(2)]

        # ---- projection packets for one t-chunk ---------------------
        def proj_packets(t4):
            sl = slice(t4 * TC, (t4 + 1) * TC)

            def p_u():
                pu = sp.tile([12, TC], F32, name="pu", tag="mm", bufs=2)
                for c in range(KC):
                    nc.tensor.matmul(pu[:], whq[:, c, :], xt[c][:, sl],
                                     start=(c == 0), stop=(c == KC - 1))
                def fin():
                    nc.scalar.activation(uz[0:12, sl], pu[:], Act.Tanh)
                return fin

            def p_a():
                for h in range(HPC):
                    pa = sp.tile([12, TC], F32, name="pa", tag="mm", bufs=2)
                    nc.tensor.matmul(pa[:], bTz[:, 12 * h:12 * h + 12],
                                     uz[:, sl], start=True, stop=True)
                    nc.vector.tensor_copy(aT[h][0:12, sl], pa[:])
                return None

            def mk_qkv(oc):
                def p_qkv():
                    pq = sp.tile([128, TC], F32, name="pq", tag="mm", bufs=2)
                    for c in range(KC):
                        nc.tensor.matmul(pq[:], wq[c][:, oc * 128:(oc + 1) * 128],
                                         xt[c][:, sl], start=(c == 0),
                                         stop=(c == KC - 1))
                    def fin():
                        if oc < 2:
                            nc.vector.tensor_copy(qz[2 * oc][0:HD, sl],
                                                  pq[0:HD, :])
                            nc.scalar.copy(qz[2 * oc + 1][HD:128, sl],
                                           pq[HD:128, :])
                        else:
                            nc.vector.tensor_copy(kt[oc - 2][:, sl], pq[:])
                    return fin
                return p_qkv

            def mk_vt(k):
                # V^T computed directly: out[t, hd] = sum_d x[t, d] wv[d, hd]
                # (lhsT = x^T block, moving = the v columns of wqkv)
                def p_vt():
                    tb = 4 * t4 + k
                    tsl = slice(tb * JB, (tb + 1) * JB)
                    pv = sp.tile([128, HPC * HD], F32, name="pv", tag="mm",
                                 bufs=2)
                    for c in range(KC):
                        nc.tensor.matmul(pv[:], xt[c][:, tsl],
                                         wq[c][:, 512:768],
                                         start=(c == 0), stop=(c == KC - 1))
                    def fin():
                        nc.vector.tensor_copy(
                            vp_all[:, :, tb, 0:HD],
                            pv[:].rearrange("p (h c) -> p h c", c=HD))
                    return fin
                return p_vt

            pk = [p_u, mk_qkv(0), mk_qkv(1), mk_qkv(2), mk_qkv(3), p_a]
            pk += [mk_vt(k) for k in range(4)]
            return pk

        # ---- attention row for one (chunk, head-pair) ---------------
        def attn_row(ic, i, feed):
            he, ho = 2 * i, 2 * i + 1
            njb = 4 * (ic + 1)
            isl = slice(ic * TC, (ic + 1) * TC)
            po_e = sp.tile([HD + 1, TC], F32, name="po_e", tag="po", bufs=2)
            po_o = sp.tile([HD + 1, TC], F32, name="po_o", tag="po", bufs=2)
            for jb in range(njb):
                if feed:
                    feed()
                jsl = slice(jb * JB, (jb + 1) * JB)
                off = max(0, jb * JB - ic * TC)
                csl = slice(ic * TC + off, (ic + 1) * TC)
                pr = sp.tile([128, 2 * TC], F32, name="pr", tag="pr", bufs=2)
                nc.tensor.matmul(pr[:, off:TC], kt[i][:, jsl],
                                 qz[he][:, csl], start=True, stop=True)
                nc.tensor.matmul(pr[:, TC + off:2 * TC], kt[i][:, jsl],
                                 qz[ho][:, csl], start=True, stop=True)
                t2 = wp.tile([128, 2 * TC], SC, name="t2", tag="t2", bufs=6)
                nc.scalar.activation(t2[:, off:2 * TC], pr[:, off:2 * TC],
                                     Act.Tanh, scale=inv_beta)
                # bias matmuls reuse the PSUM slots (WAR on the tanh)
                nc.tensor.matmul(pr[:, off:TC], uz[:, jsl],
                                 aT[he][:, csl], start=True, stop=True)
                nc.tensor.matmul(pr[:, TC + off:2 * TC], uz[:, jsl],
                                 aT[ho][:, csl], start=True, stop=True)
                nc.vector.scalar_tensor_tensor(
                    t2[:, off:2 * TC], t2[:, off:2 * TC], alpha,
                    pr[:, off:2 * TC], op0=Alu.mult, op1=Alu.add)
                ee = wp.tile([128, 2 * TC], SC, name="ee", tag="ee", bufs=6)
                nc.scalar.activation(ee[:, off:2 * TC], t2[:, off:2 * TC],
                                     Act.Exp)
                if jb >= 4 * ic:  # diagonal block: causal mask (gpsimd --
                    # the only engine with cycles to spare here)
                    nc.gpsimd.tensor_mul(ee[:, off:off + JB],
                                         ee[:, off:off + JB], tri[:])
                    nc.gpsimd.tensor_mul(ee[:, TC + off:TC + off + JB],
                                         ee[:, TC + off:TC + off + JB],
                                         tri[:])
                nc.tensor.matmul(po_e[:, off:], vp_all[:, he, jb, :],
                                 ee[:, off:TC],
                                 start=(jb == 0), stop=(jb == njb - 1))
                nc.tensor.matmul(po_o[:, off:], vp_all[:, ho, jb, :],
                                 ee[:, TC + off:2 * TC],
                                 start=(jb == 0), stop=(jb == njb - 1))
            # normalize: broadcast denominator row down 64 partitions via
            # rank-1 matmul, reciprocal, multiply
            for k, po in ((0, po_e), (1, po_o)):
                dz = dnz[k]
                nc.vector.tensor_copy(dz[0:1, :], po[HD:HD + 1, :])
                prb = sp.tile([HD, TC], F32, name="prb", tag="mm", bufs=2)
                nc.tensor.matmul(prb[:], e0z[:], dz[:], start=True, stop=True)
                rb = wp.tile([HD, TC], F32, name="rb", tag="rb", bufs=2)
                nc.vector.reciprocal_approx_fast(rb[:], prb[:])
                nc.vector.tensor_mul(ao[i][k * HD:(k + 1) * HD, isl],
                                     po[0:HD, :], rb[:])

        def outproj_packets(ic):
            sl = slice(ic * TC, (ic + 1) * TC)

            def mk(ec):
                def p_out():
                    esl = slice(ec * 128, (ec + 1) * 128)
                    pf = sp.tile([128, TC], F32, name="pf", tag="mm", bufs=2)
                    nc.tensor.matmul(pf[:], owt[0][:, esl], ao[0][:, sl],
                                     start=True, stop=False)
                    nc.tensor.matmul(pf[:], owt[1][:, esl], ao[1][:, sl],
                                     start=False, stop=True)
                    def fin():
                        fo = wp.tile([128, TC], SC, name="fo", tag="fo",
                                     bufs=4)
                        if ec % 2 == 0:
                            nc.scalar.copy(fo[:], pf[:])
                        else:
                            nc.vector.tensor_copy(fo[:], pf[:])
                        nc.sync.dma_start(out=dr["poutT"][esl, sl], in_=fo[:])
                    return fin
                return p_out
            return [mk(ec) for ec in range(D // 128)]

        # ---- main schedule ------------------------------------------
        # proj packets are emitted AFTER the attention rows they overlap
        # with: lower scheduler priority, so the critical attention chain
        # owns the engines and proj fills the gaps.
        for f in proj_packets(0):
            fin = f()
            if fin:
                fin()
        for ic in range(NTC):
            pending = list(proj_packets(ic + 1)) if ic + 1 < NTC else []
            if ic > 0:
                pending.extend(outproj_packets(ic - 1))
            it = iter(pending)
            held = [None]

            def feed():
                if held[0]:
                    held[0]()
                    held[0] = None
                f = next(it, None)
                if f:
                    held[0] = f()
            attn_row(ic, 0, feed)
            attn_row(ic, 1, feed)
            if held[0]:
                held[0]()
            for f in it:
                fin = f()
                if fin:
                    fin()
        for f in outproj_packets(NTC - 1):
            fin = f()
            if fin:
                fin()


def _build(alpha, inv_beta):
    nc = bacc.Bacc("TRN2", debug=False)
    dr = {}
    dr["xT"] = nc.dram_tensor("xT", [D, T], BF16, kind="ExternalInput").ap()
    dr["wqkvT"] = nc.dram_tensor(
        "wqkvT", [D, 3 * HPC * HD], BF16, kind="ExternalInput").ap()
    dr["whqT"] = nc.dram_tensor("whqT", [D, 12], BF16, kind="ExternalInput").ap()
    dr["bT"] = nc.dram_tensor("bT", [12, 48], BF16, kind="ExternalInput").ap()
    dr["owT"] = nc.dram_tensor(
        "owT", [HPC * HD, D], F32R, kind="ExternalInput").ap()
    dr["ident"] = nc.dram_tensor("ident", [128, 128], BF16, kind="ExternalInput").ap()
    dr["tri"] = nc.dram_tensor("tri", [128, 128], BF16, kind="ExternalInput").ap()
    dr["e0z"] = nc.dram_tensor("e0z", [128, HD], F32R, kind="ExternalInput").ap()
    dr["zz"] = nc.dram_tensor("zz", [128, TC], F32R, kind="ExternalInput").ap()
    dr["poutT"] = nc.dram_tensor("poutT", [D, T], BF16, kind="ExternalOutput").ap()
    with tile.TileContext(nc) as tc_:
        _emit(nc, tc_, dr, alpha, inv_beta)
    nc.compile()
    return nc


def _sigmoid(v):
    return 1.0 / (1.0 + np.exp(-v))


def _round_f32r(a):
    """Round fp32 -> fp32r bit pattern (11-bit mantissa, rte)."""
    u = np.ascontiguousarray(a, np.float32).view(np.uint32)
    r = (u + 0x7FF + ((u >> 12) & 1)) & np.uint32(0xFFFFF000)
    return r.view(np.float32)


def _bf(a):
    return np.ascontiguousarray(a, np.float32).astype(ml_dtypes.bfloat16)


def _host_prep(x, qkv_w, out_w, hex_w, hamming_lambda_logit, q6_w,
               transforms, transform_weights, scale_logit, sips_alpha,
               sips_beta):
    x = np.asarray(x, np.float32)
    qkv_w = np.asarray(qkv_w, np.float32)
    out_w = np.asarray(out_w, np.float32)
    hex_w = np.asarray(hex_w, np.float32)
    q6_w = np.asarray(q6_w, np.float32)
    transforms = np.asarray(transforms, np.float32)
    transform_weights = np.asarray(transform_weights, np.float32)

    lam = float(_sigmoid(np.float32(hamming_lambda_logit)))
    scale2 = float(_sigmoid(np.float32(scale_logit))) * 2.0
    alpha = float(np.asarray(sips_alpha).reshape(-1)[0])
    inv_beta = 1.0 / float(np.asarray(sips_beta).reshape(-1)[0])

    tw = np.asarray(transform_weights, np.float64) / TEMP
    w = np.exp(tw - tw.max(-1, keepdims=True))
    w = (w / w.sum(-1, keepdims=True)).astype(np.float32)      # (H, NT)
    Mh = np.einsum("ht,tde->hde", w, transforms)               # (H, 6, 6)

    whqT = _bf(np.vstack([hex_w, q6_w]).T)                     # (D, 12)
    ident = _bf(np.eye(128, dtype=np.float32))
    tri = _bf((np.arange(128)[None, :] >= np.arange(128)[:, None])
              .astype(np.float32))
    e0z_h = np.zeros((128, HD), np.float32); e0z_h[0, :] = 1.0
    bigB = np.zeros((H, 12, 12), np.float32)
    for h in range(H):
        bigB[h, :6, :6] = (lam / 2.0) * np.eye(6, dtype=np.float32)
        bigB[h, 6:, 6:] = (scale2 / 6.0) * Mh[h]

    in_maps = []
    for core in range(NCORES):
        b = core // CPB
        heads = [(core % CPB) * HPC + k for k in range(HPC)]
        rows = []
        for part in range(3):
            for h in heads:
                rows.extend(range(part * D + h * HD, part * D + (h + 1) * HD))
        wqkvT = _bf(qkv_w[rows, :].T)                          # (D, 768)
        cols = []
        for h in heads:
            cols.extend(range(h * HD, (h + 1) * HD))
        owT = _round_f32r(out_w[:, cols].T)                    # (256, D)
        bT = np.concatenate([bigB[h].T for h in heads], axis=1)  # (12, 48)
        in_maps.append({
            "xT": _bf(x[b].T),
            "wqkvT": wqkvT,
            "whqT": whqT,
            "bT": _bf(bT),
            "owT": owT,
            "ident": ident,
            "e0z": e0z_h,
            "zz": np.zeros((128, TC), np.float32),
            "tri": tri,
        })
    return in_maps, alpha, inv_beta


_CACHE = {}
LAST_RESULT = None


def kernel(**inputs):
    global LAST_RESULT
    in_maps, alpha, inv_beta = _host_prep(**inputs)
    key = (round(alpha, 9), round(inv_beta, 9))
    if key not in _CACHE:
        _CACHE[key] = _build(alpha, inv_beta)
    nc = _CACHE[key]
    res = run_bass_kernel_spmd(nc, in_maps, list(range(NCORES)))
    LAST_RESULT = res
    out = np.zeros((B, T, D), np.float32)
    for b in range(B):
        acc = np.zeros((D, T), np.float32)
        for core in range(b * CPB, (b + 1) * CPB):
            acc += np.asarray(res.results[core]["poutT"], np.float32)
        out[b] = acc.T
    return out
